# revision 20
# baseline (speedup 1.0000x reference)
"""Trainium2 Bass kernel for ContextHyperLinearSSM.

Computes out[b,:] = x[b,:] @ (WA[context[b]] * adj_xx) + u[b,:] @ (WB[context[b]] * adj_xu)

Strategy: shard the CONTEXT axis across the 8 cores (64 contexts each).
The host groups samples by context (padded to the max group size G), so each
core streams its 64 contexts' weight banks from HBM exactly once, applies the
adjacency masks on-device, and runs 3 accumulating matmuls per context
(two K=128 chunks of the A term + one for the B term).  Each sample's row is
computed by exactly one core, so the host-side unshard is a pure scatter.
"""

import sys

sys.path.insert(0, "/opt/trn_rl_repo")

import numpy as np

import concourse.bass as bass
import concourse.mybir as mybir
import concourse.tile as tile
from concourse import bacc
from concourse.bass import ts
from concourse.bass_utils import run_bass_kernel_spmd

N_CORES = 8
CT = 8  # contexts per DMA group

# matmul operand dtype: float32 (4 cyc/row) or float32r (1 cyc/row at N>=256)
MM_DT = mybir.dt.float32


def _install_profile_shim():
    """Register the NTFF profile hook that trn_boot skips when
    antenv.axon_hooks is missing from the image (profiling only)."""
    import types
    if "antenv.axon_hooks" in sys.modules:
        return
    try:
        from trn_agent_boot.trn_boot import _ntff_profile_via_ctypes
        hook = _ntff_profile_via_ctypes("/opt/axon/libaxon_pjrt.so")
    except Exception:
        hook = None
    mod = types.ModuleType("antenv.axon_hooks")
    mod.get_axon_ntff_profile_hook = lambda: hook
    mod.set_axon_ntff_profile_hook = lambda h: None
    sys.modules["antenv.axon_hooks"] = mod


def _build_program(CP, S, A, G):
    """Build the per-core Bass program. CP contexts/core, group size G."""
    f32 = mybir.dt.float32
    nc = bacc.Bacc("TRN2", target_bir_lowering=False)

    HS = S // 128  # number of 128-row K-chunks of the A-term contraction
    SO = S // 128  # number of 128-col output s-chunks
    assert S % 128 == 0 and A == 128
    NG = CP // CT
    assert CP % CT == 0
    # contexts per PSUM bank (each context needs SO*G f32 columns)
    PK = max(1, min(CT, 512 // (SO * G)))

    # weight/activation blobs are pre-laid-out by the host so every group
    # DMA is one fully contiguous span with 128-partition lines
    wa = nc.dram_tensor("wa", [NG, 128, CT, HS, S], f32,
                        kind="ExternalInput").ap()
    wb = nc.dram_tensor("wb", [NG, 128, CT, S], f32, kind="ExternalInput").ap()
    xt = nc.dram_tensor("xt", [NG, 128, CT, HS, G], f32,
                        kind="ExternalInput").ap()
    ut = nc.dram_tensor("ut", [NG, 128, CT, G], f32, kind="ExternalInput").ap()
    adj_xx = nc.dram_tensor("adj_xx", [HS, 128, S], mybir.dt.uint8,
                            kind="ExternalInput").ap()
    adj_xu = nc.dram_tensor("adj_xu", [A, S], mybir.dt.uint8,
                            kind="ExternalInput").ap()
    # transposed output blob: [group][s-partition][context][s-chunk][sample]
    out = nc.dram_tensor("out", [NG, 128, CT, SO, G], f32,
                         kind="ExternalOutput").ap()

    with tile.TileContext(nc) as tc:
        with (
            tc.tile_pool(name="const", bufs=1) as const,
            tc.tile_pool(name="w", bufs=2) as wpool,
            tc.tile_pool(name="xu", bufs=3) as xpool,
            tc.tile_pool(name="o", bufs=3) as opool,
            tc.tile_pool(name="psum", bufs=8, space="PSUM") as psum,
        ):
            # adjacency masks, cast uint8 -> f32 during the (SWDGE) DMA
            adjA0 = const.tile([128, HS, S], f32)
            nc.gpsimd.dma_start(adjA0[:], adj_xx.rearrange("h p s -> p h s"))
            adjB0 = const.tile([128, S], f32)
            nc.gpsimd.dma_start(adjB0[:], adj_xu[:])
            # funnel through the engine that consumes each mask so the
            # per-group mask-multiplies get same-engine deps (no extra
            # semaphore waits): A-masks -> DVE, B-masks -> GpSimd
            adjA = const.tile([128, HS, S], f32)
            nc.vector.tensor_copy(adjA[:], adjA0[:])
            adjB = const.tile([128, S], f32)
            nc.vector.tensor_copy(adjB[:], adjB0[:])

            rounded = MM_DT == mybir.dt.float32r
            for g in range(NG):
                cs = ts(g, CT)
                wa_t = wpool.tile([128, CT, HS, S], f32)
                nc.sync.dma_start(wa_t[:], wa[g])
                wb_t = wpool.tile([128, CT, S], f32)
                nc.sync.dma_start(wb_t[:], wb[g])
                xt_t = xpool.tile([128, CT, HS, G], f32)
                nc.sync.dma_start(xt_t[:], xt[g])
                ut_t = xpool.tile([128, CT, G], f32)
                nc.sync.dma_start(ut_t[:], ut[g])

                if rounded:
                    # fp32r consumers need fp32r-rounded producers
                    wa_m = wpool.tile([128, CT, HS, S], MM_DT, tag="wa_m")
                    wb_m = wpool.tile([128, CT, S], MM_DT, tag="wb_m")
                    xt_m = xpool.tile([128, CT, HS, G], MM_DT, tag="xt_m")
                    ut_m = xpool.tile([128, CT, G], MM_DT, tag="ut_m")
                    nc.vector.tensor_copy(xt_m[:], xt_t[:])
                    nc.vector.tensor_copy(ut_m[:], ut_t[:])
                else:
                    wa_m, wb_m, xt_m, ut_m = wa_t, wb_t, xt_t, ut_t

                # mask the weights: A on DVE, B on GpSimd (balanced rates)
                nc.vector.tensor_tensor(
                    wa_m[:], wa_t[:],
                    adjA[:, None, :, :].to_broadcast([128, CT, HS, S]),
                    mybir.AluOpType.mult)
                nc.vector.tensor_tensor(
                    wb_m[:], wb_t[:],
                    adjB[:, None, :].to_broadcast([128, CT, S]),
                    mybir.AluOpType.mult)

                # weights-stationary matmuls: out.T[sj, :, g] accumulates in
                # PSUM, PK contexts packed per bank
                out_sb = opool.tile([128, CT, SO, G], f32)
                for c0 in range(0, CT, PK):
                    npk = min(PK, CT - c0)
                    ps = psum.tile([128, PK * SO * G], f32)
                    for ci in range(npk):
                        c = c0 + ci
                        for j in range(SO):
                            pslice = ps[:, (ci * SO + j) * G:
                                        (ci * SO + j) * G + G]
                            for h in range(HS):
                                nc.tensor.matmul(
                                    pslice,
                                    lhsT=wa_m[:, c, h, ts(j, 128)],
                                    rhs=xt_m[:, c, h, :],
                                    start=(h == 0), stop=False)
                            nc.tensor.matmul(
                                pslice,
                                lhsT=wb_m[:, c, ts(j, 128)],
                                rhs=ut_m[:, c, :],
                                start=False, stop=True)
                    nc.scalar.copy(
                        out_sb[:, c0:c0 + npk, :, :].rearrange(
                            "p c j g -> p (c j g)"),
                        ps[:, :npk * SO * G])
                nc.scalar.dma_start(out[g], out_sb[:])

    nc.compile()
    return nc


def kernel(x, u, WA, WB, adj_xx, adj_xu, context, _trace=False):
    B, S = x.shape
    _, A = u.shape
    C = WA.shape[0]
    assert C % N_CORES == 0
    CP = C // N_CORES

    # ---- host-side shard: group samples by context --------------------
    context = np.asarray(context)
    cnt = np.bincount(context, minlength=C)
    G = int(cnt.max())
    G = max(4, ((G + 3) // 4) * 4)
    order = np.argsort(context, kind="stable")
    starts = np.zeros(C + 1, np.int64)
    starts[1:] = np.cumsum(cnt)
    j = np.arange(G)
    valid = j[None, :] < cnt[:, None]                      # [C, G]
    pos = starts[:-1, None] + np.minimum(j[None, :],
                                         np.maximum(cnt[:, None] - 1, 0))
    gidx = order[pos]                                      # [C, G]

    Xp = np.asarray(x, np.float32)[gidx]                   # [C, G, S]
    Up = np.asarray(u, np.float32)[gidx]                   # [C, G, A]
    XpT = np.ascontiguousarray(Xp.transpose(0, 2, 1))      # [C, S, G]
    UpT = np.ascontiguousarray(Up.transpose(0, 2, 1))      # [C, A, G]

    WA = np.ascontiguousarray(WA, np.float32)
    WB = np.ascontiguousarray(WB, np.float32)
    adjxx_u8 = np.ascontiguousarray(adj_xx).view(np.uint8).reshape(S // 128, 128, S)
    adjxu_u8 = np.ascontiguousarray(adj_xu).view(np.uint8)

    HS = S // 128
    NG = CP // CT
    in_maps = []
    for k in range(N_CORES):
        sl = slice(k * CP, (k + 1) * CP)
        # relayout: group DMAs become contiguous [128, CT*HS*S] spans
        wa_k = np.ascontiguousarray(
            WA[sl].reshape(NG, CT, HS, 128, S).transpose(0, 3, 1, 2, 4))
        wb_k = np.ascontiguousarray(
            WB[sl].reshape(NG, CT, 128, S).transpose(0, 2, 1, 3))
        xt_k = np.ascontiguousarray(
            XpT[sl].reshape(NG, CT, HS, 128, G).transpose(0, 3, 1, 2, 4))
        ut_k = np.ascontiguousarray(
            UpT[sl].reshape(NG, CT, 128, G).transpose(0, 2, 1, 3))
        in_maps.append({
            "wa": wa_k,
            "wb": wb_k,
            "xt": xt_k,
            "ut": ut_k,
            "adj_xx": adjxx_u8,
            "adj_xu": adjxu_u8,
        })

    if _trace:
        _install_profile_shim()
    nc = _build_program(CP, S, A, G)
    res = run_bass_kernel_spmd(nc, in_maps, core_ids=list(range(N_CORES)),
                               trace=_trace)

    # device output is out.T blobs [NG, 128, CT, SO, G] -> [CP, G, S]
    outs = [
        r["out"].transpose(0, 2, 4, 3, 1).reshape(CP, G, S)
        for r in res.results
    ]
    Out_all = np.concatenate(outs, axis=0)                 # [C, G, S]
    out_full = np.zeros((B, S), np.float32)
    out_full[gidx[valid]] = Out_all[valid]

    if _trace:
        return out_full, res
    return out_full


# revision 35
# speedup vs baseline: 1.4299x; 1.4299x over previous
"""Trainium2 Bass kernel for ContextHyperLinearSSM.

Computes out[b,:] = x[b,:] @ (WA[context[b]] * adj_xx) + u[b,:] @ (WB[context[b]] * adj_xu)

Strategy: shard the CONTEXT axis across the 8 cores (64 contexts each).
The host groups samples by context (padded to the max group size G), so each
core streams its 64 contexts' weight banks from HBM exactly once, applies the
adjacency masks on-device, and runs 3 accumulating matmuls per context
(two K=128 chunks of the A term + one for the B term).  Each sample's row is
computed by exactly one core, so the host-side unshard is a pure scatter.
"""

import sys

sys.path.insert(0, "/opt/trn_rl_repo")

import numpy as np

import concourse.bass as bass
import concourse.mybir as mybir
import concourse.tile as tile
from concourse import bacc
from concourse.bass import ts
from concourse.bass_utils import run_bass_kernel_spmd

N_CORES = 8
CT = 8  # contexts per DMA group

# matmul operand dtype: float32 (4 cyc/row) or float32r (1 cyc/row at N>=256)
MM_DT = mybir.dt.float32


def _install_profile_shim():
    """Register the NTFF profile hook that trn_boot skips when
    antenv.axon_hooks is missing from the image (profiling only)."""
    import types
    if "antenv.axon_hooks" in sys.modules:
        return
    try:
        from trn_agent_boot.trn_boot import _ntff_profile_via_ctypes
        hook = _ntff_profile_via_ctypes("/opt/axon/libaxon_pjrt.so")
    except Exception:
        hook = None
    mod = types.ModuleType("antenv.axon_hooks")
    mod.get_axon_ntff_profile_hook = lambda: hook
    mod.set_axon_ntff_profile_hook = lambda h: None
    sys.modules["antenv.axon_hooks"] = mod


def _build_program(CP, S, A, G):
    """Build the per-core Bass program. CP contexts/core, group size G."""
    f32 = mybir.dt.float32
    nc = bacc.Bacc("TRN2", target_bir_lowering=False)

    HS = S // 128  # number of 128-row K-chunks of the A-term contraction
    assert S % 128 == 0 and A == 128
    NG = CP // CT
    assert CP % CT == 0
    # PSUM context packing: FF contexts along the free dim of a bank and
    # (for G <= 64) two partition slots at 0/64 -- matmul outputs may only
    # base at partition 0/32/64.  CPT contexts share one bank; T banks/group.
    FF = max(1, min(CT, 512 // S))
    PSL = 2 if G <= 64 else 1
    CPT = min(CT, PSL * FF)
    T = -(-CT // CPT)
    assert T * CPT == CT, (CT, FF, PSL, CPT)

    # weight/activation blobs are pre-laid-out by the host so every group
    # DMA is one fully contiguous span with 128-partition lines
    wa = nc.dram_tensor("wa", [NG, 128, CT, HS, S], f32,
                        kind="ExternalInput").ap()
    wb = nc.dram_tensor("wb", [NG, 128, CT, S], f32, kind="ExternalInput").ap()
    xt = nc.dram_tensor("xt", [NG, 128, CT, HS, G], f32,
                        kind="ExternalInput").ap()
    ut = nc.dram_tensor("ut", [NG, 128, CT, G], f32, kind="ExternalInput").ap()
    adj_xx = nc.dram_tensor("adj_xx", [HS, 128, S], mybir.dt.uint8,
                            kind="ExternalInput").ap()
    adj_xu = nc.dram_tensor("adj_xu", [A, S], mybir.dt.uint8,
                            kind="ExternalInput").ap()
    # output blob: [group][partition][bank][context-half][s]
    out = nc.dram_tensor("out", [NG, 128, T, FF, S], f32,
                         kind="ExternalOutput").ap()

    with tile.TileContext(nc) as tc:
        with (
            tc.tile_pool(name="const", bufs=1) as const,
            tc.tile_pool(name="w", bufs=2) as wpool,
            tc.tile_pool(name="xu", bufs=3) as xpool,
            tc.tile_pool(name="o", bufs=3) as opool,
            tc.tile_pool(name="psum", bufs=8, space="PSUM") as psum,
        ):
            # adjacency masks, cast uint8 -> f32 during the (SWDGE) DMA
            adjA0 = const.tile([128, HS, S], f32)
            nc.gpsimd.dma_start(adjA0[:], adj_xx.rearrange("h p s -> p h s"))
            adjB0 = const.tile([128, S], f32)
            nc.gpsimd.dma_start(adjB0[:], adj_xu[:])
            # A-mask consumed by DVE via a same-engine funnel copy.
            # B-mask goes to GpSimd, which cannot handle stride-0
            # (broadcast) APs -- materialize it replicated CT times.
            adjA = const.tile([128, HS, S], f32)
            nc.vector.tensor_copy(adjA[:], adjA0[:])
            adjB = const.tile([128, CT, S], f32)
            nc.vector.tensor_copy(
                adjB[:], adjB0[:, None, :].to_broadcast([128, CT, S]))

            rounded = MM_DT == mybir.dt.float32r
            for g in range(NG):
                cs = ts(g, CT)
                wa_t = wpool.tile([128, CT, HS, S], f32)
                nc.sync.dma_start(wa_t[:], wa[g])
                wb_t = wpool.tile([128, CT, S], f32)
                nc.sync.dma_start(wb_t[:], wb[g])
                xt_t = xpool.tile([128, CT, HS, G], f32)
                nc.sync.dma_start(xt_t[:], xt[g])
                ut_t = xpool.tile([128, CT, G], f32)
                nc.sync.dma_start(ut_t[:], ut[g])

                if rounded:
                    # fp32r consumers need fp32r-rounded producers
                    wa_m = wpool.tile([128, CT, HS, S], MM_DT, tag="wa_m")
                    wb_m = wpool.tile([128, CT, S], MM_DT, tag="wb_m")
                    xt_m = xpool.tile([128, CT, HS, G], MM_DT, tag="xt_m")
                    ut_m = xpool.tile([128, CT, G], MM_DT, tag="ut_m")
                    nc.vector.tensor_copy(xt_m[:], xt_t[:])
                    nc.vector.tensor_copy(ut_m[:], ut_t[:])
                else:
                    wa_m, wb_m, xt_m, ut_m = wa_t, wb_t, xt_t, ut_t

                # mask the weights: A on DVE, B on GpSimd (balanced rates)
                nc.vector.tensor_tensor(
                    wa_m[:], wa_t[:],
                    adjA[:, None, :, :].to_broadcast([128, CT, HS, S]),
                    mybir.AluOpType.mult)
                nc.gpsimd.tensor_tensor(
                    wb_m[:], wb_t[:], adjB[:], mybir.AluOpType.mult)

                # x-stationary matmuls streaming masked weights (N=S rows);
                # all CT contexts of a group pack into ONE psum bank:
                # context c -> partition slot c//FF, free half c%FF
                ps_tiles = [psum.tile([128, FF * S], f32, tag="ps",
                                      name=f"ps_{g}_{t}")
                            for t in range(T)]
                for c in range(CT):
                    t, r2 = divmod(c, CPT)
                    sl, cf = divmod(r2, FF)
                    pslice = ps_tiles[t][sl * 64:sl * 64 + G,
                                         cf * S:cf * S + S]
                    for h in range(HS):
                        nc.tensor.matmul(
                            pslice,
                            lhsT=xt_m[:, c, h, :],
                            rhs=wa_m[:, c, h, :],
                            start=(h == 0), stop=False)
                    nc.tensor.matmul(
                        pslice,
                        lhsT=ut_m[:, c, :],
                        rhs=wb_m[:, c, :],
                        start=False, stop=True)
                out_sb = opool.tile([128, T, FF, S], f32)
                for t in range(T):
                    nc.scalar.copy(
                        out_sb[:, t].rearrange("p f s -> p (f s)"),
                        ps_tiles[t][:])
                nc.scalar.dma_start(out[g], out_sb[:])

    nc.compile()
    return nc


def kernel(x, u, WA, WB, adj_xx, adj_xu, context, _trace=False):
    B, S = x.shape
    _, A = u.shape
    C = WA.shape[0]
    assert C % N_CORES == 0
    CP = C // N_CORES

    # ---- host-side shard: group samples by context --------------------
    context = np.asarray(context)
    cnt = np.bincount(context, minlength=C)
    G = int(cnt.max())
    G = max(4, ((G + 3) // 4) * 4)
    order = np.argsort(context, kind="stable")
    starts = np.zeros(C + 1, np.int64)
    starts[1:] = np.cumsum(cnt)
    j = np.arange(G)
    valid = j[None, :] < cnt[:, None]                      # [C, G]
    pos = starts[:-1, None] + np.minimum(j[None, :],
                                         np.maximum(cnt[:, None] - 1, 0))
    gidx = order[pos]                                      # [C, G]

    Xp = np.asarray(x, np.float32)[gidx]                   # [C, G, S]
    Up = np.asarray(u, np.float32)[gidx]                   # [C, G, A]
    XpT = np.ascontiguousarray(Xp.transpose(0, 2, 1))      # [C, S, G]
    UpT = np.ascontiguousarray(Up.transpose(0, 2, 1))      # [C, A, G]

    WA = np.ascontiguousarray(WA, np.float32)
    WB = np.ascontiguousarray(WB, np.float32)
    adjxx_u8 = np.ascontiguousarray(adj_xx).view(np.uint8).reshape(S // 128, 128, S)
    adjxu_u8 = np.ascontiguousarray(adj_xu).view(np.uint8)

    HS = S // 128
    NG = CP // CT
    in_maps = []
    for k in range(N_CORES):
        sl = slice(k * CP, (k + 1) * CP)
        # relayout: group DMAs become contiguous [128, CT*HS*S] spans
        wa_k = np.ascontiguousarray(
            WA[sl].reshape(NG, CT, HS, 128, S).transpose(0, 3, 1, 2, 4))
        wb_k = np.ascontiguousarray(
            WB[sl].reshape(NG, CT, 128, S).transpose(0, 2, 1, 3))
        xt_k = np.ascontiguousarray(
            XpT[sl].reshape(NG, CT, HS, 128, G).transpose(0, 3, 1, 2, 4))
        ut_k = np.ascontiguousarray(
            UpT[sl].reshape(NG, CT, 128, G).transpose(0, 2, 1, 3))
        in_maps.append({
            "wa": wa_k,
            "wb": wb_k,
            "xt": xt_k,
            "ut": ut_k,
            "adj_xx": adjxx_u8,
            "adj_xu": adjxu_u8,
        })

    if _trace:
        _install_profile_shim()
    nc = _build_program(CP, S, A, G)
    res = run_bass_kernel_spmd(nc, in_maps, core_ids=list(range(N_CORES)),
                               trace=_trace)

    # device output blobs [NG, 128, T, FF, S] -> [CP, G, S].
    # context c in a group lives at bank t=c//CPT, partition slot
    # sl=(c%CPT)//FF (64-aligned), free half cf=c%FF.
    o0 = res.results[0]["out"]
    NGo, _, To, FFo, _ = o0.shape
    PSLo = CP // (NGo * To * FFo)
    outs = []
    for r in res.results:
        v = r["out"].reshape(NGo, PSLo, 128 // PSLo, To, FFo, S)[:, :, :G]
        # axes (g, sl, gg, t, cf, s) -> (g, t, sl, cf, gg, s)
        v = v.transpose(0, 3, 1, 4, 2, 5).reshape(CP, G, S)
        outs.append(v)
    Out_all = np.concatenate(outs, axis=0)                 # [C, G, S]
    out_full = np.zeros((B, S), np.float32)
    out_full[gidx[valid]] = Out_all[valid]

    if _trace:
        return out_full, res
    return out_full


# revision 38
# speedup vs baseline: 1.8857x; 1.3188x over previous
"""Trainium2 Bass kernel for ContextHyperLinearSSM.

Computes out[b,:] = x[b,:] @ (WA[context[b]] * adj_xx) + u[b,:] @ (WB[context[b]] * adj_xu)

Strategy: shard the CONTEXT axis across the 8 cores (64 contexts each).
The host groups samples by context (padded to the max group size G), so each
core streams its 64 contexts' weight banks from HBM exactly once, applies the
adjacency masks on-device, and runs 3 accumulating matmuls per context
(two K=128 chunks of the A term + one for the B term).  Each sample's row is
computed by exactly one core, so the host-side unshard is a pure scatter.
"""

import sys

sys.path.insert(0, "/opt/trn_rl_repo")

import numpy as np

import concourse.bass as bass
import concourse.mybir as mybir
import concourse.tile as tile
from concourse import bacc
from concourse.bass import ts
from concourse.bass_utils import run_bass_kernel_spmd

N_CORES = 8
CT = 8  # contexts per DMA group

# matmul operand dtype: float32 (4 cyc/row) or float32r (1 cyc/row at N>=256)
MM_DT = mybir.dt.float32


def _install_profile_shim():
    """Register the NTFF profile hook that trn_boot skips when
    antenv.axon_hooks is missing from the image (profiling only)."""
    import types
    if "antenv.axon_hooks" in sys.modules:
        return
    try:
        from trn_agent_boot.trn_boot import _ntff_profile_via_ctypes
        hook = _ntff_profile_via_ctypes("/opt/axon/libaxon_pjrt.so")
    except Exception:
        hook = None
    mod = types.ModuleType("antenv.axon_hooks")
    mod.get_axon_ntff_profile_hook = lambda: hook
    mod.set_axon_ntff_profile_hook = lambda h: None
    sys.modules["antenv.axon_hooks"] = mod


def _build_program(CP, S, A, G):
    """Build the per-core Bass program. CP contexts/core, group size G."""
    f32 = mybir.dt.float32
    nc = bacc.Bacc("TRN2", target_bir_lowering=False)

    HS = S // 128  # number of 128-row K-chunks of the A-term contraction
    assert S % 128 == 0 and A == 128
    NG = CP // CT
    assert CP % CT == 0
    # PSUM context packing: FF contexts along the free dim of a bank and
    # (for G <= 64) two partition slots at 0/64 -- matmul outputs may only
    # base at partition 0/32/64.  CPT contexts share one bank; T banks/group.
    FF = max(1, min(CT, 512 // S))
    PSL = 2 if G <= 64 else 1
    CPT = min(CT, PSL * FF)
    T = -(-CT // CPT)
    assert T * CPT == CT, (CT, FF, PSL, CPT)

    # weight/activation blobs are pre-laid-out by the host so every group
    # DMA is one fully contiguous span with 128-partition lines
    wa = nc.dram_tensor("wa", [NG, 128, CT, HS, S], f32,
                        kind="ExternalInput").ap()
    wb = nc.dram_tensor("wb", [NG, 128, CT, S], f32, kind="ExternalInput").ap()
    xt = nc.dram_tensor("xt", [NG, 128, CT, HS, G], f32,
                        kind="ExternalInput").ap()
    ut = nc.dram_tensor("ut", [NG, 128, CT, G], f32, kind="ExternalInput").ap()
    adj_xx = nc.dram_tensor("adj_xx", [HS, 128, S], mybir.dt.uint8,
                            kind="ExternalInput").ap()
    adj_xu = nc.dram_tensor("adj_xu", [A, S], mybir.dt.uint8,
                            kind="ExternalInput").ap()
    # output blob: [group][partition][bank][context-half][s]
    out = nc.dram_tensor("out", [NG, 128, T, FF, S], f32,
                         kind="ExternalOutput").ap()

    with tile.TileContext(nc) as tc:
        with (
            tc.tile_pool(name="const", bufs=1) as const,
            tc.tile_pool(name="w", bufs=3) as wpool,
            tc.tile_pool(name="xu", bufs=3) as xpool,
            tc.tile_pool(name="o", bufs=3) as opool,
            tc.tile_pool(name="psum", bufs=8, space="PSUM") as psum,
        ):
            # adjacency masks, cast uint8 -> f32 during the (SWDGE) DMA
            adjA0 = const.tile([128, HS, S], f32)
            nc.gpsimd.dma_start(adjA0[:], adj_xx.rearrange("h p s -> p h s"))
            adjB0 = const.tile([128, S], f32)
            nc.gpsimd.dma_start(adjB0[:], adj_xu[:])
            # funnel both masks through DVE so the per-group mask
            # multiplies carry same-engine deps only
            adjA = const.tile([128, HS, S], f32)
            nc.vector.tensor_copy(adjA[:], adjA0[:])
            adjB = const.tile([128, S], f32)
            nc.vector.tensor_copy(adjB[:], adjB0[:])

            rounded = MM_DT == mybir.dt.float32r
            for g in range(NG):
                cs = ts(g, CT)
                wa_t = wpool.tile([128, CT, HS, S], f32)
                nc.sync.dma_start(wa_t[:], wa[g])
                wb_t = wpool.tile([128, CT, S], f32)
                nc.sync.dma_start(wb_t[:], wb[g])
                xt_t = xpool.tile([128, CT, HS, G], f32)
                nc.sync.dma_start(xt_t[:], xt[g])
                ut_t = xpool.tile([128, CT, G], f32)
                nc.sync.dma_start(ut_t[:], ut[g])

                if rounded:
                    # fp32r consumers need fp32r-rounded producers
                    wa_m = wpool.tile([128, CT, HS, S], MM_DT, tag="wa_m")
                    wb_m = wpool.tile([128, CT, S], MM_DT, tag="wb_m")
                    xt_m = xpool.tile([128, CT, HS, G], MM_DT, tag="xt_m")
                    ut_m = xpool.tile([128, CT, G], MM_DT, tag="ut_m")
                    nc.vector.tensor_copy(xt_m[:], xt_t[:])
                    nc.vector.tensor_copy(ut_m[:], ut_t[:])
                else:
                    wa_m, wb_m, xt_m, ut_m = wa_t, wb_t, xt_t, ut_t

                # mask the weights on DVE; B first (it gates each context's
                # last matmul), and A split in halves so the PE can start
                # on the first CT/2 contexts earlier
                nc.vector.tensor_tensor(
                    wb_m[:], wb_t[:],
                    adjB[:, None, :].to_broadcast([128, CT, S]),
                    mybir.AluOpType.mult)
                CH = CT // 2
                for half in range(2):
                    hs = slice(half * CH, (half + 1) * CH)
                    nc.vector.tensor_tensor(
                        wa_m[:, hs], wa_t[:, hs],
                        adjA[:, None, :, :].to_broadcast([128, CH, HS, S]),
                        mybir.AluOpType.mult)

                # x-stationary matmuls streaming masked weights (N=S rows);
                # all CT contexts of a group pack into ONE psum bank:
                # context c -> partition slot c//FF, free half c%FF
                ps_tiles = [psum.tile([128, FF * S], f32, tag="ps",
                                      name=f"ps_{g}_{t}")
                            for t in range(T)]
                for c in range(CT):
                    t, r2 = divmod(c, CPT)
                    sl, cf = divmod(r2, FF)
                    pslice = ps_tiles[t][sl * 64:sl * 64 + G,
                                         cf * S:cf * S + S]
                    for h in range(HS):
                        nc.tensor.matmul(
                            pslice,
                            lhsT=xt_m[:, c, h, :],
                            rhs=wa_m[:, c, h, :],
                            start=(h == 0), stop=False)
                    nc.tensor.matmul(
                        pslice,
                        lhsT=ut_m[:, c, :],
                        rhs=wb_m[:, c, :],
                        start=False, stop=True)
                out_sb = opool.tile([128, T, FF, S], f32)
                for t in range(T):
                    nc.scalar.copy(
                        out_sb[:, t].rearrange("p f s -> p (f s)"),
                        ps_tiles[t][:])
                nc.scalar.dma_start(out[g], out_sb[:])

    nc.compile()
    return nc


def kernel(x, u, WA, WB, adj_xx, adj_xu, context, _trace=False):
    B, S = x.shape
    _, A = u.shape
    C = WA.shape[0]
    assert C % N_CORES == 0
    CP = C // N_CORES

    # ---- host-side shard: group samples by context --------------------
    context = np.asarray(context)
    cnt = np.bincount(context, minlength=C)
    G = int(cnt.max())
    G = max(4, ((G + 3) // 4) * 4)
    order = np.argsort(context, kind="stable")
    starts = np.zeros(C + 1, np.int64)
    starts[1:] = np.cumsum(cnt)
    j = np.arange(G)
    valid = j[None, :] < cnt[:, None]                      # [C, G]
    pos = starts[:-1, None] + np.minimum(j[None, :],
                                         np.maximum(cnt[:, None] - 1, 0))
    gidx = order[pos]                                      # [C, G]

    Xp = np.asarray(x, np.float32)[gidx]                   # [C, G, S]
    Up = np.asarray(u, np.float32)[gidx]                   # [C, G, A]
    XpT = np.ascontiguousarray(Xp.transpose(0, 2, 1))      # [C, S, G]
    UpT = np.ascontiguousarray(Up.transpose(0, 2, 1))      # [C, A, G]

    WA = np.ascontiguousarray(WA, np.float32)
    WB = np.ascontiguousarray(WB, np.float32)
    adjxx_u8 = np.ascontiguousarray(adj_xx).view(np.uint8).reshape(S // 128, 128, S)
    adjxu_u8 = np.ascontiguousarray(adj_xu).view(np.uint8)

    HS = S // 128
    NG = CP // CT
    in_maps = []
    for k in range(N_CORES):
        sl = slice(k * CP, (k + 1) * CP)
        # relayout: group DMAs become contiguous [128, CT*HS*S] spans
        wa_k = np.ascontiguousarray(
            WA[sl].reshape(NG, CT, HS, 128, S).transpose(0, 3, 1, 2, 4))
        wb_k = np.ascontiguousarray(
            WB[sl].reshape(NG, CT, 128, S).transpose(0, 2, 1, 3))
        xt_k = np.ascontiguousarray(
            XpT[sl].reshape(NG, CT, HS, 128, G).transpose(0, 3, 1, 2, 4))
        ut_k = np.ascontiguousarray(
            UpT[sl].reshape(NG, CT, 128, G).transpose(0, 2, 1, 3))
        in_maps.append({
            "wa": wa_k,
            "wb": wb_k,
            "xt": xt_k,
            "ut": ut_k,
            "adj_xx": adjxx_u8,
            "adj_xu": adjxu_u8,
        })

    if _trace:
        _install_profile_shim()
    nc = _build_program(CP, S, A, G)
    res = run_bass_kernel_spmd(nc, in_maps, core_ids=list(range(N_CORES)),
                               trace=_trace)

    # device output blobs [NG, 128, T, FF, S] -> [CP, G, S].
    # context c in a group lives at bank t=c//CPT, partition slot
    # sl=(c%CPT)//FF (64-aligned), free half cf=c%FF.
    o0 = res.results[0]["out"]
    NGo, _, To, FFo, _ = o0.shape
    PSLo = CP // (NGo * To * FFo)
    outs = []
    for r in res.results:
        v = r["out"].reshape(NGo, PSLo, 128 // PSLo, To, FFo, S)[:, :, :G]
        # axes (g, sl, gg, t, cf, s) -> (g, t, sl, cf, gg, s)
        v = v.transpose(0, 3, 1, 4, 2, 5).reshape(CP, G, S)
        outs.append(v)
    Out_all = np.concatenate(outs, axis=0)                 # [C, G, S]
    out_full = np.zeros((B, S), np.float32)
    out_full[gidx[valid]] = Out_all[valid]

    if _trace:
        return out_full, res
    return out_full
